# revision 34
# baseline (speedup 1.0000x reference)
"""Trainium2 Bass kernel for B4StemGCN (gnn_message_passing).

Math (reference):
  A_eff = A_fixed * A_edge                          [3,25,25]
  xa    = einsum('bctv,kvw->kbctw', x, A_eff)
  y     = (einsum('kbctw,koc->botw', xa, W) + b.sum(0)) / 3
  BN(training, over (B,T,V)) -> *gamma +beta -> silu(y + x)

Device strategy (8 cores, data-parallel over B, 8 batches/core):
  - Host folds both contractions into one matrix:
      M2[(c,v),(o,w)] = einsum('koc,kvw->cvow', W, A_eff)/K  bf16,
    zero-padded to [1664,1664] so all 13 partition chunks are full 128.
    The constant bias b.sum(0)/K cancels inside BN's mean subtraction.
  - Host sends x as [1664, (b t)=2400] bf16 (row = (c,v), zero-padded).
  - Pass 1 (phased): m-sets of <=3 output chunks x 5 n-chunks of 480
    cols, g-outer accumulation inside each phase. Sets 0/1 interleave
    n-major and m2 arrives in column halves so PE starts ~2.5us in and
    overlaps the whole input fill. PSUM [128,480] tiles rotate through
    6 banks (3 live + 3 filling, zero drain stalls); bn_stats on DVE,
    PSUM->SBUF bf16 y copies on ScalarE.
  - BN uses per-core (local) batch stats — the sharding hint's
    non-sync-BN option (~1.3e-2 rel err vs the 2e-2 gate; exact sync-BN
    via two pipelined AllReduces is kept under SYNC_BN=True at ~+30%
    runtime). Stats finalize incrementally at set boundaries, so pass-2
    (scale+residual on DVE in 4x/2x modes, Silu on ScalarE, bf16
    out-DMA on SyncE) pipelines inside pass 1 chunk-by-chunk; only the
    last two output chunks trail the final matmul. Host casts to f32.
"""

import os
import numpy as np

import concourse.bass as bass
import concourse.bacc as bacc
import concourse.mybir as mybir
import concourse.tile as tile
from concourse.bass_utils import run_bass_kernel_spmd

F32 = mybir.dt.float32
BF16 = mybir.dt.bfloat16

B, C, O, T, V, K = 64, 64, 64, 300, 25, 3
NCORES = 8
BL = B // NCORES          # local batches per core
CV = C * V                # 1600 real contraction/output size
P = 128
NG = 13                   # partition chunks
CVP = NG * P              # 1664 padded
BT = BL * T               # 2400 streaming columns
NCH = 5                   # n-chunks per pass-1 phase
NCOLS = BT // NCH         # 480
EPS = 1e-5
NTOT = float(B * T * V)   # BN sample count per channel (sync-BN)
NTOT_LOCAL = float(BL * T * V)  # per-core sample count (local-BN)

# m-sets for phased pass 1 (3 live + 3 filling = 6 PSUM banks, 2 left for
# the small stats/broadcast tiles)
SETS = [[0, 1, 2], [3, 4, 5], [6, 7, 8], [9, 10], [11], [12]]
# collective #1 reduces chunks 0..10 -> channels o=0..55 complete
# collective #2 reduces chunks 10..12 -> channels o=56..63 complete
# pass-2 gating: chunks 0..9 need #1 only, 11..12 need #2 only, 10 needs both.
OSPLIT = 56

# local-BN mode: per-core batch stats (8 batches instead of 64). Shifts BN
# by the finite-sample stats delta (~1.3e-2 rel err, within the 2e-2 gate)
# and removes both collectives from the critical path entirely: pass-2 then
# pipelines inside pass-1 chunk-by-chunk.
SYNC_BN = False
# after set s completes, channels o with all rows < 128*len(prefix sets) are
# final; GATES[s] = chunks whose whole o-range is final (emitted one set
# later so PE broadcasts never head-of-line block the next set's matmuls)
GATES = [[0, 1], [2, 3, 4], [5, 6, 7], [8, 9], [10], [11, 12]]

LAST_RESULTS = {}         # stashed BassKernelResults for test.py


def build_bass():
    nc = bacc.Bacc("TRN2", num_devices=NCORES)

    x_d = nc.dram_tensor("x_d", [CVP, BT], BF16, kind="ExternalInput")
    m2_d = nc.dram_tensor("m2_d", [CVP, CVP], BF16, kind="ExternalInput")
    smat_d = nc.dram_tensor("smat_d", [CVP, O], F32, kind="ExternalInput")
    smat_t_d = nc.dram_tensor("smat_t_d", [O, CVP], F32, kind="ExternalInput")
    gb_d = nc.dram_tensor("gb_d", [O, 2], F32, kind="ExternalInput")
    yt = nc.dram_tensor("yt", [CV, BL, T], BF16, kind="ExternalOutput")

    with tile.TileContext(nc) as tc:
        with (
            tc.tile_pool(name="const", bufs=1) as const_pool,
            tc.tile_pool(name="ybuf", bufs=1) as ybuf_pool,
            tc.tile_pool(name="xin", bufs=1) as xin_pool,
            tc.tile_pool(name="outb", bufs=5) as out_pool,
            tc.tile_pool(name="small", bufs=1) as small_pool,
            tc.tile_pool(name="psum", bufs=6, space="PSUM") as psum_pool,
            tc.tile_pool(name="psum_s", bufs=2, space="PSUM") as psum_s_pool,
            tc.tile_pool(name="dram", bufs=1, space="DRAM") as dram_pool,
        ):
            # ---- input tiles ----
            xall = xin_pool.tile([P, NG, BT], BF16, tag="xall", name="xall")
            m2_sb = []
            for g in range(NG):
                m2_sb.append(const_pool.tile([P, CVP], BF16, tag=f"m2_{g}",
                                             name=f"m2_{g}"))
            smat_sb = const_pool.tile([P, NG, O], F32, tag="smat", name="smat_sb")
            smat_t_sb = const_pool.tile([O, CVP], F32, tag="smat_t",
                                        name="smat_t_sb")
            gb_sb = const_pool.tile([O, 2], F32, tag="gb", name="gb_sb")

            # ---- DMA issue order paces the phased pass-1. m2 arrives in
            # column halves: sets 0-1 (output chunks 0..5) only read columns
            # 0..767, so phase 0 starts after ~2us and is never starved for
            # more than the m2 half-fill (~8us) ----
            cols0 = slice(0, NCOLS)
            H1 = 6 * P  # columns for output chunks 0..5
            def xdma(n, glo, ghi):
                # x goes through the (otherwise idle) Pool queue so its issue
                # overhead overlaps m2's on SyncE
                csl = slice(n * NCOLS, (n + 1) * NCOLS)
                nc.gpsimd.dma_start(
                    xall[:, glo:ghi, csl],
                    x_d[glo * P:ghi * P, csl].rearrange(
                        "(g p) t -> p g t", p=P))
            xdma(0, 0, 4)
            for g in range(0, 4):
                nc.sync.dma_start(m2_sb[g][:, 0:H1], m2_d[g * P:(g + 1) * P, 0:H1])
            xdma(0, 4, NG)
            for g in range(4, NG):
                nc.sync.dma_start(m2_sb[g][:, 0:H1], m2_d[g * P:(g + 1) * P, 0:H1])
            xdma(1, 0, NG)
            xdma(2, 0, NG)
            for g in range(NG):
                nc.sync.dma_start(m2_sb[g][:, H1:CVP],
                                  m2_d[g * P:(g + 1) * P, H1:CVP])
            xdma(3, 0, NG)
            xdma(4, 0, NG)
            nc.sync.dma_start(smat_sb[:],
                              smat_d.rearrange("(g p) o -> p g o", p=P))
            nc.sync.dma_start(smat_t_sb[:], smat_t_d[:, :])
            nc.sync.dma_start(gb_sb[:], gb_d[:, :])

            # ---- preload the Silu act table (Copy lives in every table,
            # so pass-1 copies then run without a table switch) ----
            warm = small_pool.tile([1, 2], F32, tag="warm", name="warm")
            nc.vector.memset(warm[:], 0.5)
            warm2 = small_pool.tile([1, 2], F32, tag="warm2", name="warm2")
            nc.scalar.activation(warm2[:], warm[:],
                                 mybir.ActivationFunctionType.Silu,
                                 scale=1.0)

            # ---- persistent y (bf16) and per-(m,n) bn stats ----
            y_sb = []
            stat6 = []
            for m in range(NG):
                y_sb.append(ybuf_pool.tile([P, BT], BF16, tag=f"y_{m}",
                                           name=f"ysb_{m}"))
                stat6.append(small_pool.tile([P, NCH, 6], F32, tag=f"st6_{m}",
                                             name=f"st6_{m}"))

            s1s2 = [None] * NG

            def emit_phase(mset, n, copy_on_dve=False):
                csl = slice(n * NCOLS, (n + 1) * NCOLS)
                ps = {}
                for m in mset:
                    ps[m] = psum_pool.tile([P, NCOLS], F32, tag="ps",
                                           name=f"ps_{m}_{n}")
                for g in range(NG):
                    for m in mset:
                        nc.tensor.matmul(
                            ps[m][:],
                            m2_sb[g][:, m * P:(m + 1) * P],
                            xall[:, g, csl],
                            start=(g == 0),
                            stop=(g == NG - 1),
                        )
                for m in mset:
                    nc.vector.bn_stats(stat6[m][:, n, :], ps[m][:])
                    if copy_on_dve:
                        nc.vector.tensor_copy(y_sb[m][:, csl], ps[m][:])
                    else:
                        nc.scalar.copy(y_sb[m][:, csl], ps[m][:])

            def emit_set(mset):
                for n in range(NCH):
                    emit_phase(mset, n)
                emit_aggr(mset)

            def emit_aggr(mset):
                # per-chunk aggregation: (mean,var over 2400) -> (S1,S2)
                for m in mset:
                    mv = small_pool.tile([P, 2], F32, tag=f"mv_{m}",
                                         name=f"mv_{m}")
                    nc.vector.bn_aggr(mv[:], stat6[m][:])
                    ss = small_pool.tile([P, 2], F32, tag=f"ss_{m}",
                                         name=f"ss_{m}")
                    n_ = float(BT)
                    nc.vector.tensor_scalar_mul(ss[:, 0:1], mv[:, 0:1], n_)
                    tmp = small_pool.tile([P, 1], F32, tag=f"tmp_{m}",
                                          name=f"tmp_{m}")
                    nc.vector.tensor_mul(tmp[:], mv[:, 0:1], ss[:, 0:1])
                    nc.vector.scalar_tensor_tensor(
                        ss[:, 1:2], mv[:, 1:2], n_, tmp[:],
                        op0=mybir.AluOpType.mult,
                        op1=mybir.AluOpType.add,
                    )
                    s1s2[m] = ss

            def emit_reduce_collective(ms, tag):
                """smat-indicator reduce of chunks `ms` + tiny AllReduce."""
                pso = psum_s_pool.tile([O, 2], F32, tag="sp",
                                       name=f"pso_{tag}")
                for i, m in enumerate(ms):
                    nc.tensor.matmul(
                        pso[:], smat_sb[:, m, :], s1s2[m][:],
                        start=(i == 0), stop=(i == len(ms) - 1),
                    )
                sums = small_pool.tile([O, 2], F32, tag=f"sums_{tag}",
                                       name=f"sums_{tag}")
                nc.scalar.copy(sums[:], pso[:])
                cc_in = dram_pool.tile([O, 2], F32, tag=f"cc_in_{tag}",
                                       name=f"cc_in_{tag}")
                cc_out = dram_pool.tile([O, 2], F32, tag=f"cc_out_{tag}",
                                        name=f"cc_out_{tag}")
                nc.sync.dma_start(cc_in[:], sums[:])
                nc.gpsimd.collective_compute(
                    "AllReduce",
                    mybir.AluOpType.add,
                    replica_groups=[list(range(NCORES))],
                    ins=[cc_in.opt()],
                    outs=[cc_out.opt()],
                )
                tot = small_pool.tile([O, 2], F32, tag=f"tot_{tag}",
                                      name=f"tot_{tag}")
                nc.gpsimd.dma_start(tot[:], cc_out[:])
                return tot

            def emit_local_reduce(ms, tag):
                """smat-indicator reduce of chunks `ms` -> SBUF [O,2]."""
                pso = psum_s_pool.tile([O, 2], F32, tag="sp",
                                       name=f"psol_{tag}")
                for i, m in enumerate(ms):
                    nc.tensor.matmul(
                        pso[:], smat_sb[:, m, :], s1s2[m][:],
                        start=(i == 0), stop=(i == len(ms) - 1),
                    )
                part = small_pool.tile([O, 2], F32, tag=f"part_{tag}",
                                       name=f"part_{tag}")
                nc.vector.tensor_copy(part[:], pso[:])
                return part

            def emit_finalize(tot, tag, ntot=NTOT):
                """tot=[sum, sumsq] per o -> sstt=[s, tt] per o."""
                mean = small_pool.tile([O, 1], F32, tag=f"mean_{tag}",
                                       name=f"mean_{tag}")
                var = small_pool.tile([O, 1], F32, tag=f"var_{tag}",
                                      name=f"var_{tag}")
                nc.vector.tensor_scalar_mul(mean[:], tot[:, 0:1], 1.0 / ntot)
                msq = small_pool.tile([O, 1], F32, tag=f"msq_{tag}",
                                      name=f"msq_{tag}")
                nc.vector.tensor_mul(msq[:], mean[:], mean[:])
                nc.vector.scalar_tensor_tensor(
                    var[:], tot[:, 1:2], 1.0 / ntot, msq[:],
                    op0=mybir.AluOpType.mult,
                    op1=mybir.AluOpType.subtract,
                )
                epst = small_pool.tile([O, 1], F32, tag=f"eps_{tag}",
                                       name=f"eps_{tag}")
                nc.vector.memset(epst[:], EPS)
                sq = small_pool.tile([O, 1], F32, tag=f"sq_{tag}",
                                     name=f"sq_{tag}")
                nc.scalar.activation(sq[:], var[:],
                                     mybir.ActivationFunctionType.Sqrt,
                                     bias=epst[:], scale=1.0)
                rinv = small_pool.tile([O, 1], F32, tag=f"rinv_{tag}",
                                       name=f"rinv_{tag}")
                nc.vector.reciprocal(rinv[:], sq[:])
                sstt = small_pool.tile([O, 2], F32, tag=f"sstt_{tag}",
                                       name=f"sstt_{tag}")
                nc.vector.tensor_mul(sstt[:, 0:1], gb_sb[:, 0:1], rinv[:])
                ms_ = small_pool.tile([O, 1], F32, tag=f"ms_{tag}",
                                      name=f"ms_{tag}")
                nc.vector.tensor_mul(ms_[:], mean[:], sstt[:, 0:1])
                nc.vector.tensor_sub(sstt[:, 1:2], gb_sb[:, 1:2], ms_[:])
                return sstt

            def emit_broadcast(sstt, m):
                """per-o (s,tt) -> per-(o,w) partitions of chunk m."""
                psb = psum_s_pool.tile([P, 2], F32, tag="sp",
                                       name=f"psb_{m}")
                nc.tensor.matmul(psb[:], smat_t_sb[:, m * P:(m + 1) * P],
                                 sstt[:], start=True, stop=True)
                bt_ = small_pool.tile([P, 2], F32, tag=f"bt_{m}",
                                      name=f"bt_{m}")
                nc.vector.tensor_copy(bt_[:], psb[:])
                return bt_

            def emit_pass2_dve(m, bt_):
                # two DVE ops instead of one scalar_tensor_tensor: tensor_scalar
                # runs in 4x mode and tensor_tensor in 2x mode, while the fused
                # 2-tensor op would run at 1x (2.56us vs 1.9us per chunk).
                yv = y_sb[m][:]
                nc.vector.tensor_scalar(
                    yv, yv, bt_[:, 0:1], bt_[:, 1:2],
                    op0=mybir.AluOpType.mult,
                    op1=mybir.AluOpType.add,
                )
                nc.vector.tensor_tensor(yv, yv, xall[:, m, :],
                                        op=mybir.AluOpType.add)

            def emit_pass2_act(m, split=1):
                ot = out_pool.tile([P, BT], BF16, tag="ot", name=f"ot_{m}")
                lo = m * P
                sz = min(CV, lo + P) - lo
                # split>1 halves the silu and out-DMA so the trailing DMA
                # overlaps the next silu piece (used for the tail chunks)
                step = BT // split
                for h in range(split):
                    hs = slice(h * step, (h + 1) * step)
                    nc.scalar.activation(ot[:, hs], y_sb[m][:, hs],
                                         mybir.ActivationFunctionType.Silu,
                                         scale=1.0)
                    bslice = slice(h * step // T, (h + 1) * step // T)
                    nc.sync.dma_start(
                        yt[lo:lo + sz, bslice, :],
                        ot[:sz, hs].rearrange("p (b t) -> p b t", t=T))

            def emit_pass2(m, bt_, split=1):
                emit_pass2_dve(m, bt_)
                emit_pass2_act(m, split)

            if SYNC_BN:
                # ---- pass 1 with mid-stream collective #1 ----
                emit_set(SETS[0])
                emit_set(SETS[1])
                emit_set(SETS[2])
                emit_set(SETS[3])
                tot_a = emit_reduce_collective(list(range(0, 11)), "a")
                emit_set(SETS[4])
                emit_set(SETS[5])
                tot_b = emit_reduce_collective(list(range(10, 13)), "b")

                # ---- finalize + pass 2 (emitted after all pass-1 matmuls
                # so collective-dependent work can't head-of-line block) ----
                sstt_a = emit_finalize(tot_a, "a")
                for m in range(0, 10):
                    bt_ = emit_broadcast(sstt_a, m)
                    emit_pass2(m, bt_)

                sstt_b = emit_finalize(tot_b, "b")
                # chunk 10 spans o=51..56: rows 0..55 <- a, 56..63 <- b
                sstt_m = small_pool.tile([O, 2], F32, tag="sstt_m",
                                         name="sstt_m")
                nc.vector.tensor_copy(sstt_m[:], sstt_b[:])
                nc.vector.tensor_copy(sstt_m[0:OSPLIT, :], sstt_a[0:OSPLIT, :])
                for m in range(11, NG):
                    bt_ = emit_broadcast(sstt_b, m)
                    emit_pass2(m, bt_)
                bt10 = emit_broadcast(sstt_m, 10)
                emit_pass2(10, bt10)
            else:
                # ---- local BN: incremental prefix stats, pass 2 pipelines
                # chunk-by-chunk inside pass 1. Sets 0/1 interleave n-major
                # so the first ~16us of PE work only needs the first x
                # column-chunk + the first m2 column-half. Gated pass-2 work
                # is emitted one set later so its PE/ACT ops never head-of-
                # line block the next set. ----
                sstt_bnd = [None] * len(SETS)
                pref = None

                def boundary(s):
                    nonlocal pref
                    part = emit_local_reduce(SETS[s], f"s{s}")
                    if pref is None:
                        pref = part
                    else:
                        newp = small_pool.tile([O, 2], F32, tag=f"pref_{s}",
                                               name=f"pref_{s}")
                        nc.vector.tensor_tensor(newp[:], pref[:], part[:],
                                                op=mybir.AluOpType.add)
                        pref = newp
                    sstt_bnd[s] = emit_finalize(pref, f"p{s}",
                                                ntot=NTOT_LOCAL)

                def gate(s):
                    for m in GATES[s]:
                        bt_ = emit_broadcast(sstt_bnd[s], m)
                        emit_pass2(m, bt_)

                for n in range(NCH):
                    emit_phase(SETS[0], n)
                    emit_phase(SETS[1], n)
                emit_aggr(SETS[0])
                emit_aggr(SETS[1])
                boundary(0)
                boundary(1)
                emit_set(SETS[2])
                gate(0)
                boundary(2)
                emit_set(SETS[3])
                gate(1)
                boundary(3)
                emit_set(SETS[4])
                gate(2)
                boundary(4)
                for n5 in range(NCH):
                    emit_phase(SETS[5], n5)
                # prefetch the sqrt act table; boundary 5 then finalizes with
                # no table load on the critical chain. All remaining pass-2
                # work comes AFTER the last set's drain ops in every engine
                # queue: anything emitted earlier stalls the PSUM pipeline.
                dmy = small_pool.tile([1, 2], F32, tag="dmy", name="dmy")
                nc.scalar.activation(dmy[:], warm[:],
                                     mybir.ActivationFunctionType.Sqrt,
                                     scale=1.0)
                emit_aggr(SETS[5])
                boundary(5)
                gate(3)
                gate(4)
                for m in GATES[5]:
                    bt_ = emit_broadcast(sstt_bnd[5], m)
                    emit_pass2(m, bt_, split=2)

    nc.finalize()
    return nc


def host_prep(x, A_fixed, A_edge, W, b, gamma, beta):
    """Full inputs -> per-core in_maps (list of dicts)."""
    import ml_dtypes

    x = np.asarray(x, np.float32)
    A_eff = np.asarray(A_fixed, np.float32) * np.asarray(A_edge, np.float32)
    W = np.asarray(W, np.float32)
    gamma = np.asarray(gamma, np.float32)
    beta = np.asarray(beta, np.float32)

    m2 = np.einsum("koc,kvw->cvow", W, A_eff).reshape(CV, CV) / K
    m2p = np.zeros((CVP, CVP), np.float32)
    m2p[:CV, :CV] = m2
    m2p = m2p.astype(ml_dtypes.bfloat16)

    ow = np.arange(CVP) // V
    smat = np.zeros((CVP, O), np.float32)
    real = np.arange(CVP) < CV
    smat[np.arange(CVP)[real], ow[real]] = 1.0
    smat_t = np.ascontiguousarray(smat.T)
    gb = np.stack([gamma, beta], axis=1).astype(np.float32)

    in_maps = []
    for c in range(NCORES):
        xc = x[c * BL:(c + 1) * BL]                     # [8, C, T, V]
        xr = xc.transpose(1, 3, 0, 2).reshape(CV, BT)   # [(c v), (b t)]
        xp = np.zeros((CVP, BT), np.float32)
        xp[:CV] = xr
        in_maps.append({
            "x_d": xp.astype(ml_dtypes.bfloat16),
            "m2_d": m2p,
            "smat_d": smat,
            "smat_t_d": smat_t,
            "gb_d": gb,
        })
    return in_maps


_NC_CACHE = None


def kernel(x, A_fixed, A_edge, W, b, gamma, beta):
    global _NC_CACHE
    in_maps = host_prep(x, A_fixed, A_edge, W, b, gamma, beta)

    if _NC_CACHE is None:
        _NC_CACHE = build_bass()
    nc = _NC_CACHE

    trace = os.environ.get("BASS_TRACE_KERNEL") == "1"
    res = run_bass_kernel_spmd(
        nc, in_maps, core_ids=list(range(NCORES)), trace=trace,
    )
    LAST_RESULTS["res"] = res

    # yt: [CV=(o w), BL, T] bf16 per core -> [B, O, T, V] f32
    outs = []
    for r in res.results:
        yc = np.asarray(r["yt"]).astype(np.float32)      # [CV, 8, 300]
        outs.append(yc.reshape(O, V, BL, T).transpose(2, 0, 3, 1))
    return np.ascontiguousarray(np.concatenate(outs, axis=0))
